# revision 12
# baseline (speedup 1.0000x reference)
"""Trainium2 Bass kernel for bare-Coulomb GNN message passing.

potential[t] = 0.5 * sum_{pairs} 1/r * charges[s]  (both directions):
  - host: directed contributions (t <- s) are atom-sharded across 8 cores
    (12500 atoms/core), assigned to the SBUF partition owning t, t-sorted,
    source-sorted within each (partition, target) run.  The charges table is
    packed 16 atoms per 256-byte row (6250 rows, 1.6 MB), so the dma_gather
    row index is s//16 (int16, no bank sectioning) and the gather stays in a
    small hot HBM region; a one-hot fp16 mask selects atom s%16 from each
    gathered row.
  - device per core: big (8192-idx) dma_gather calls pull 16-atom rows per
    contribution; DVE masks+reduces to the 4 target channels, computes
    0.5/r * q, and runs a per-partition chained prefix scan per channel; the
    prefix is streamed to DRAM; the 98 per-atom run-end prefix values per
    partition are gathered back (per-partition indirect DMA) and differenced
    to yield per-atom sums.
  - host: concatenate the 8 per-core outputs.
"""
import os
import numpy as np

N_ATOMS = 100000
N_CHANNELS = 4
NCORES = 8
AT_CORE = N_ATOMS // NCORES          # 12500
P = 128
R_AT = 98                            # atoms per partition row-grid (98*128=12544)
GRID = P * R_AT                      # 12544
ATOMS_PER_ROW = 16
TROWS = 6256                         # ceil(100000/16)=6250, padded
CALL_NI = int(os.environ.get("KCALL_NI", "8192"))   # indices per dma_gather call
NQUEUES = int(os.environ.get("KNQ", "4"))
SPC_G = CALL_NI // P                 # slots per partition per gather call
SPC = 256                            # slots per partition per chunk
GCALLS = SPC // SPC_G                # gather calls per chunk
PAD_DIST = 65504.0                   # fp16 max; 1/x ~ 1.5e-5 ~ 0

_CACHE = {}


def _preprocess(neighbor_indices, neighbor_distances):
    """Host-side index-metadata layout. Returns per-core input arrays + consts."""
    idx = np.asarray(neighbor_indices).astype(np.int64)
    dist = np.asarray(neighbor_distances).astype(np.float32)
    t = np.concatenate([idx[:, 0], idx[:, 1]])
    s = np.concatenate([idx[:, 1], idx[:, 0]])
    dd = np.concatenate([dist, dist])

    core = t // AT_CORE
    tl = t - core * AT_CORE                       # local atom id
    p = tl // R_AT                                # owning partition
    r = tl - p * R_AT
    srow = (s // ATOMS_PER_ROW).astype(np.int16)  # table row (< 6250)
    ssub = (s % ATOMS_PER_ROW).astype(np.int8)    # atom within row

    # per (core, p) counts -> uniform stream length S (SPC-aligned)
    gidx = core * P + p
    cnt_p = np.bincount(gidx, minlength=NCORES * P)
    S = int(((int(cnt_p.max()) + SPC - 1) // SPC) * SPC)
    ncalls = S // SPC_G
    nchunks = S // SPC

    # order contributions by (core, p, t, s); position within (core,p)
    order = np.lexsort((s, tl, p, core))
    co, po, srw, ssb, ddo, tlo, ro = (core[order], p[order], srow[order],
                                      ssub[order], dd[order], tl[order], r[order])
    g = co * P + po
    grp_starts = np.concatenate([[0], np.cumsum(cnt_p)[:-1]])
    slot = np.arange(len(g)) - grp_starts[g]      # slot within partition stream

    # per (core, p, r) cumulative end counts -> boundary row/col in the
    # prefix tensor, which is laid out as 256B rows of 16 slots:
    # [P * (S16+1), 64]; row p*(S16+1) is a zero row.
    S16 = S // 16
    g3 = (co * P + po) * R_AT + ro
    cnt3 = np.bincount(g3, minlength=NCORES * P * R_AT) \
             .reshape(NCORES, P, R_AT)
    endcnt = np.cumsum(cnt3, axis=2)              # inclusive
    rows_local = np.where(endcnt == 0, 0, 1 + (endcnt - 1) // 16)
    colm = np.where(endcnt == 0, 15, (endcnt - 1) % 16)   # [NCORES, P, R_AT]

    per_core = []
    W16 = CALL_NI // 16
    BJQ = 49
    BCALL_NI = BJQ * P                            # 6272
    BW16 = BCALL_NI // 16                         # 392
    for k in range(NCORES):
        m = co == k
        pk, sk, ssk, dk, slk = po[m], srw[m], ssb[m], ddo[m], slot[m]
        src = np.zeros((P, S), dtype=np.int16)
        dts = np.full((P, S), PAD_DIST, dtype=np.float16)
        msk = np.zeros((P, S, ATOMS_PER_ROW), dtype=np.float16)
        src[pk, slk] = sk
        dts[pk, slk] = dk.astype(np.float16)
        msk[pk, slk, ssk] = 1.0
        # wrapped idx tiles: descriptor i -> partition i%128, slot i//128
        L = src.reshape(P, ncalls, SPC_G).transpose(1, 2, 0) \
               .reshape(ncalls, CALL_NI)
        w16 = L.reshape(ncalls, W16, 16).transpose(0, 2, 1)   # [q, 16, W16]
        idxw = np.tile(w16, (1, 8, 1)).transpose(1, 0, 2) \
                 .reshape(P, ncalls * W16)
        # boundary gather: 2 partition-halves x 2 calls of 49 boundary cols
        rl = rows_local[k]                        # [P, R_AT]
        bcalls = []
        for k2 in range(2):
            for jq in range(2):
                i = np.arange(BCALL_NI)
                jj, pp = i // P, i % P
                j = jq * BJQ + jj
                Lb = np.where(
                    (pp // 64 == k2),
                    (pp % 64) * (S16 + 1) + rl[pp, j], 0).astype(np.int16)
                w16b = Lb.reshape(BW16, 16).T     # [16, 392]
                bcalls.append(np.tile(w16b, (8, 1)))
        bidxw = np.concatenate(bcalls, axis=1)    # [P, 4*BW16]
        bmask = np.zeros((P, R_AT, 16), dtype=np.float16)
        bmask[np.arange(P)[:, None], np.arange(R_AT)[None, :], colm[k]] = 1.0
        per_core.append({
            "idx": idxw,
            "dist": dts,
            "mask": msk.reshape(P, S * ATOMS_PER_ROW),
            "bidx": bidxw,
            "bmask": bmask.reshape(P, R_AT * 16),
        })
    consts = {"S": S, "ncalls": ncalls, "nchunks": nchunks}
    return per_core, consts


def _pad_table(charges):
    ch = np.zeros((TROWS * ATOMS_PER_ROW, N_CHANNELS), dtype=np.float32)
    ch[:N_ATOMS] = np.asarray(charges, dtype=np.float32)
    return ch.reshape(TROWS, ATOMS_PER_ROW * N_CHANNELS)   # [6256, 64]


def _build_bass(S, ncalls, nchunks):
    import concourse.bacc as bacc
    import concourse.tile as tile
    import concourse.bass as bass
    from concourse import mybir
    bisect = os.environ.get("KBISECT", "full")

    NJ = R_AT                        # 98 boundary values per partition
    W16 = CALL_NI // 16
    A = ATOMS_PER_ROW
    S16 = S // 16
    BJQ = 49
    BCALL_NI = BJQ * P               # 6272
    BW16 = BCALL_NI // 16            # 392

    nc = bacc.Bacc("TRN2", target_bir_lowering=False, debug=False,
                   num_devices=NCORES, num_swdge_queues=NQUEUES)
    table = nc.dram_tensor("table", [TROWS, 64], mybir.dt.float32,
                           kind="ExternalInput").ap()
    idx_d = nc.dram_tensor("idx", [P, ncalls * W16], mybir.dt.int16,
                           kind="ExternalInput").ap()
    dist_d = nc.dram_tensor("dist", [P, S], mybir.dt.float16,
                            kind="ExternalInput").ap()
    mask_d = nc.dram_tensor("mask", [P, S * A], mybir.dt.float16,
                            kind="ExternalInput").ap()
    bidx_d = nc.dram_tensor("bidx", [P, 4 * BW16], mybir.dt.int16,
                            kind="ExternalInput").ap()
    bmask_d = nc.dram_tensor("bmask", [P, NJ * 16], mybir.dt.float16,
                             kind="ExternalInput").ap()
    # prefix stream as 256B rows of 16 slots; row p*(S16+1) is a zero row
    prefix = nc.dram_tensor("prefix", [P * (S16 + 1), 64],
                            mybir.dt.float32, kind="Internal").ap()
    out_d = nc.dram_tensor("out", [GRID, N_CHANNELS], mybir.dt.float32,
                           kind="ExternalOutput").ap()

    prefix3 = prefix.rearrange("(p s) x -> p s x", p=P)
    out3 = out_d.rearrange("(p r) c -> p r c", p=P)

    with tile.TileContext(nc) as tc:
        with tc.tile_pool(name="idxp", bufs=2) as idxp, \
             tc.tile_pool(name="gp", bufs=2) as gp, \
             tc.tile_pool(name="mp", bufs=2) as mp, \
             tc.tile_pool(name="wp", bufs=2) as wp, \
             tc.tile_pool(name="vp", bufs=2) as vp, \
             tc.tile_pool(name="pfp", bufs=2) as pfp, \
             tc.tile_pool(name="persist", bufs=1) as pers:

            zt = pers.tile([P, 1, 64], mybir.dt.float32)
            nc.gpsimd.memset(zt[:], 0.0)
            nc.sync.dma_start(prefix3[:, 0:1, :], zt[:])

            bmk = pers.tile([P, NJ, 16], mybir.dt.float16)
            nc.sync.dma_start(bmk[:].rearrange("p j m -> p (j m)"), bmask_d[:])

            prev_pf = None
            for c in range(nchunks):
                it = idxp.tile([P, GCALLS * W16], mybir.dt.int16, tag="it")
                nc.sync.dma_start(
                    it[:], idx_d[:, c * GCALLS * W16:(c + 1) * GCALLS * W16])
                g = gp.tile([P, SPC, 64], mybir.dt.float32, tag="g")
                if "nogather" in bisect:
                    nc.gpsimd.memset(g[:], 1.0)
                else:
                    for ci in range(GCALLS):
                        nc.gpsimd.dma_gather(
                            out_ap=g[:, ci * SPC_G:(ci + 1) * SPC_G, :],
                            in_ap=table[:],
                            idxs_ap=it[:, ci * W16:(ci + 1) * W16],
                            num_idxs=CALL_NI, num_idxs_reg=CALL_NI,
                            elem_size=64, single_packet=False,
                            queue_num=ci % NQUEUES,
                        )
                mk = mp.tile([P, SPC, A], mybir.dt.float16, tag="mk")
                nc.sync.dma_start(
                    mk[:].rearrange("p s a -> p (s a)"),
                    mask_d[:, c * SPC * A:(c + 1) * SPC * A])
                # select atom s%16 from each row: g *= onehot, then sum over m
                g4 = g[:].rearrange("p s (m c) -> p s m c", c=N_CHANNELS)
                nc.vector.tensor_tensor(
                    out=g4, in0=g4,
                    in1=mk[:, :, :, None].to_broadcast([P, SPC, A, N_CHANNELS]),
                    op=mybir.AluOpType.mult)
                q = vp.tile([P, SPC, N_CHANNELS], mybir.dt.float32, tag="q")
                gT = g[:].rearrange("p s (m c) -> p s c m", c=N_CHANNELS)
                nc.vector.tensor_reduce(
                    out=q[:], in_=gT, axis=mybir.AxisListType.X,
                    op=mybir.AluOpType.add)
                dt_ = wp.tile([P, SPC], mybir.dt.float16, tag="dt")
                nc.sync.dma_start(dt_[:], dist_d[:, c * SPC:(c + 1) * SPC])
                wt = wp.tile([P, SPC], mybir.dt.float32, tag="wt")
                nc.vector.reciprocal(wt[:], dt_[:])
                nc.vector.scalar_tensor_tensor(
                    out=q[:], in0=q[:], scalar=0.5,
                    in1=wt[:, :, None].to_broadcast([P, SPC, N_CHANNELS]),
                    op0=mybir.AluOpType.mult, op1=mybir.AluOpType.mult,
                )
                pf = pfp.tile([P, SPC, N_CHANNELS], mybir.dt.float32, tag="pf")
                for ch in range(N_CHANNELS):
                    init = 0.0 if prev_pf is None \
                        else prev_pf[:, SPC - 1, ch:ch + 1]
                    nc.vector.tensor_tensor_scan(
                        out=pf[:, :, ch], data0=q[:, :, ch], data1=q[:, :, ch],
                        initial=init,
                        op0=mybir.AluOpType.add, op1=mybir.AluOpType.bypass,
                    )
                prev_pf = pf
                # 256 slots * 4ch = 16 rows of 64 fp32
                nc.sync.dma_start(
                    prefix3[:, 1 + c * (SPC // 16):1 + (c + 1) * (SPC // 16), :],
                    pf[:].rearrange("p (a b) c -> p a (b c)", a=SPC // 16))

            et = pers.tile([P, NJ, N_CHANNELS], mybir.dt.float32)
            if "noj" in bisect:
                nc.gpsimd.memset(et[:], 0.0)
            else:
                for k2 in range(2):
                    for jq in range(2):
                        cidx = k2 * 2 + jq
                        bt = idxp.tile([P, BW16], mybir.dt.int16, tag="bt")
                        nc.sync.dma_start(
                            bt[:], bidx_d[:, cidx * BW16:(cidx + 1) * BW16])
                        bb = gp.tile([P, BJQ, 64], mybir.dt.float32, tag="bb")
                        nc.gpsimd.dma_gather(
                            out_ap=bb[:],
                            in_ap=prefix[k2 * 64 * (S16 + 1):
                                         (k2 + 1) * 64 * (S16 + 1), :],
                            idxs_ap=bt[:],
                            num_idxs=BCALL_NI, num_idxs_reg=BCALL_NI,
                            elem_size=64, single_packet=False,
                            queue_num=cidx % NQUEUES,
                        )
                        sl = slice(64 * k2, 64 * (k2 + 1))
                        jsl = slice(jq * BJQ, (jq + 1) * BJQ)
                        bb4 = bb[sl].rearrange("p j (m c) -> p j m c",
                                               c=N_CHANNELS)
                        nc.vector.tensor_tensor(
                            out=bb4, in0=bb4,
                            in1=bmk[sl, jsl, :, None].to_broadcast(
                                [64, BJQ, 16, N_CHANNELS]),
                            op=mybir.AluOpType.mult)
                        bbT = bb[sl].rearrange("p j (m c) -> p j c m",
                                               c=N_CHANNELS)
                        nc.vector.tensor_reduce(
                            out=et[sl, jsl, :], in_=bbT,
                            axis=mybir.AxisListType.X, op=mybir.AluOpType.add)
            etf = et[:].rearrange("p j c -> p (j c)")
            dt2 = pers.tile([P, NJ * N_CHANNELS], mybir.dt.float32)
            nc.vector.tensor_copy(dt2[:, 0:4], etf[:, 0:4])
            nc.vector.tensor_tensor(
                out=dt2[:, 4:], in0=etf[:, 4:], in1=etf[:, 0:NJ * 4 - 4],
                op=mybir.AluOpType.subtract)
            d2v = dt2[:].rearrange("p (r c) -> p r c", c=N_CHANNELS)
            nc.sync.dma_start(out3[:, :, :], d2v)
    nc.finalize()
    return nc


class _Runner:
    def __init__(self, nc, n_cores):
        import jax
        from jax.sharding import Mesh, PartitionSpec
        try:
            from jax.experimental.shard_map import shard_map
        except Exception:
            from jax.sharding import shard_map
        from concourse import mybir
        from concourse.bass2jax import (_bass_exec_p, partition_id_tensor,
                                        install_neuronx_cc_hook)
        install_neuronx_cc_hook()
        self.jax = jax
        self.n_cores = n_cores
        pname = nc.partition_id_tensor.name if nc.partition_id_tensor else None
        in_names, out_names, out_avals, zero_outs = [], [], [], []
        for alloc in nc.m.functions[0].allocations:
            if not isinstance(alloc, mybir.MemoryLocationSet):
                continue
            name = alloc.memorylocations[0].name
            if alloc.kind == "ExternalInput":
                if name != pname:
                    in_names.append(name)
            elif alloc.kind == "ExternalOutput":
                shape = tuple(alloc.tensor_shape)
                dtype = mybir.dt.np(alloc.dtype)
                out_names.append(name)
                out_avals.append(jax.core.ShapedArray(shape, dtype))
                zero_outs.append(np.zeros(shape, dtype))
        self.in_names, self.out_names = in_names, out_names
        self.out_avals, self.zero_outs = out_avals, zero_outs
        n_params, n_outs = len(in_names), len(out_names)
        all_in = list(in_names) + list(out_names)
        if pname is not None:
            all_in.append(pname)

        def _body(*args):
            operands = list(args)
            if pname is not None:
                operands.append(partition_id_tensor())
            outs = _bass_exec_p.bind(
                *operands, out_avals=tuple(out_avals), in_names=tuple(all_in),
                out_names=tuple(out_names), lowering_input_output_aliases=(),
                sim_require_finite=False, sim_require_nnan=False, nc=nc)
            return tuple(outs)

        devices = jax.devices()[:n_cores]
        mesh = Mesh(np.asarray(devices), ("core",))
        in_specs = (PartitionSpec("core"),) * (n_params + n_outs)
        out_specs = (PartitionSpec("core"),) * n_outs
        self.fn = jax.jit(
            shard_map(_body, mesh=mesh, in_specs=in_specs,
                      out_specs=out_specs, check_rep=False),
            keep_unused=True)

    def run(self, in_maps):
        jax = self.jax
        concat_in = [
            np.concatenate([np.asarray(in_maps[c][n]) for c in range(self.n_cores)], axis=0)
            for n in self.in_names]
        concat_zeros = [
            np.zeros((self.n_cores * z.shape[0], *z.shape[1:]), z.dtype)
            for z in self.zero_outs]
        dargs = [jax.device_put(a) for a in concat_in + concat_zeros]
        outs = self.fn(*dargs)
        jax.block_until_ready(outs)
        res = []
        for c in range(self.n_cores):
            d = {}
            for i, n in enumerate(self.out_names):
                a = np.asarray(outs[i]).reshape(self.n_cores, *self.out_avals[i].shape)
                d[n] = a[c]
            res.append(d)
        return res


def _make_in_maps(charges, per_core):
    tab = _pad_table(charges)
    return [{"table": tab, "idx": pc["idx"], "dist": pc["dist"],
             "mask": pc["mask"], "bidx": pc["bidx"], "bmask": pc["bmask"]}
            for pc in per_core]


def kernel(charges, cell, positions, neighbor_indices, neighbor_distances):
    per_core, consts = _preprocess(neighbor_indices, neighbor_distances)
    key = (consts["S"], consts["ncalls"])
    if key not in _CACHE:
        nc = _build_bass(consts["S"], consts["ncalls"], consts["nchunks"])
        _CACHE[key] = _Runner(nc, NCORES)
    runner = _CACHE[key]
    in_maps = _make_in_maps(charges, per_core)
    res = runner.run(in_maps)
    out = np.concatenate([res[k]["out"][:AT_CORE] for k in range(NCORES)], axis=0)
    return out.astype(np.float32)
